# revision 27
# baseline (speedup 1.0000x reference)
"""Trainium2 Bass kernel for nn_NodeProcessor (GNN message passing).

Strategy (8 NeuronCores, SPMD, no collectives):
  - Host sorts edges by destination node and shards NODES (6250/core);
    each core receives exactly the edges destined to its node shard, so no
    cross-core reduction is needed.
  - Edge features are shipped in fp8 e3m4 (4 mantissa bits; inputs are
    N(0,1) so the range fits easily), halving edge DMA vs bf16.
  - Segment-sum per 128-node tile as a sequence of 128-edge-chunk matmuls
    accumulating in PSUM: agg_T[f, n] += sum_e E[e, f] * S[e, n].  The
    one-hot S matrices are PREBUILT ON HOST in fp8 and streamed with the
    edge data (one interleaved record per chunk: 128B edges + 32B window-S),
    so no on-device compare ops are needed.  Chunk 0 of each tile uses a
    full-width host-built S0 (start=True clears the accumulator); later
    chunks accumulate a 32-wide window at a host-baked column offset.
  - MLP batched per 7-tile group: h1_T = relu(W1.T @ [x_T; agg_T] + b1) in
    896-node moving batches; h2 node-major (h1 slices stationary) into two
    PSUM banks (4+3 tiles); bias b2 via a rank-1 ones-row matmul.
  - LayerNorm reads h2 straight from PSUM: grouped bn_stats ([P,4,128] /
    [P,3,128]), per-tile bn_aggr, batched sqrt/reciprocal, then a per-tile
    fused normalize on ACT (Identity with per-partition scale=rstd,
    bias=-mu*rstd), batched gamma multiply (Pool) and residual add (DVE)
    over the whole group; the residual (x + ln_b) is folded on host.

Matmul inputs are fp8e3 (scatter) / bf16 (MLP); accumulation is f32 in
PSUM; LayerNorm statistics and the residual add are f32.
"""

import os
import sys

import numpy as np

for _p in ("/opt/trn_rl_repo", "/root/.axon_site/_ro/trn_rl_repo"):
    if os.path.isdir(_p) and _p not in sys.path:
        sys.path.insert(0, _p)

import ml_dtypes

import concourse.bacc as bacc
import concourse.bass as bass
import concourse.tile as tile
from concourse import mybir
from concourse.bass_utils import run_bass_kernel_spmd

BF16 = ml_dtypes.bfloat16
E3 = ml_dtypes.float8_e3m4

N_NODES = 50000
N_EDGES = 600000
D = 128           # node/edge feature dim
H = 256           # hidden dim
NCORE = 8
NSHARD = N_NODES // NCORE      # 6250 real nodes per core
P = 128                        # partition / tile size
NT = 49                        # node tiles per core (49*128 = 6272 >= 6250)
G = 7                          # tiles per group
NGRP = NT // G                 # 7 groups
NPAD = NT * P
L = 32                         # edge chunks per DMA load
W = 32                         # scatter window width
R = D + W                      # bytes per chunk record row (fp8)
LN_EPS = 1e-5


def _prep_host(x, edge_index, edge_attr, W1, b1, W2, b2, ln_g, ln_b):
    """Sort/shard/pack all inputs; prebuild one-hot S matrices in fp8."""
    j = np.asarray(edge_index[1], dtype=np.int64)
    perm = np.argsort(j, kind="stable")
    js = j[perm]

    ea8 = np.asarray(edge_attr, dtype=E3)
    x = np.asarray(x, dtype=np.float32)
    ln_b = np.asarray(ln_b, dtype=np.float32)

    bounds = np.searchsorted(js, np.arange(NCORE + 1) * NSHARD)

    core_info = []
    for c in range(NCORE):
        es, ee = bounds[c], bounds[c + 1]
        jl = js[es:ee] - c * NSHARD           # local node id, 0..6249
        rows = perm[es:ee]                    # rows into edge_attr
        cnt = np.bincount(jl // P, minlength=NT)  # edges per tile
        ch = -(-cnt // P)                     # ceil chunks per tile
        tile_perm = np.argsort(ch, kind="stable")   # ascending chunk count
        # light tiles first: the first group runs while the PE p-state
        # is still ramping and the edge stream is cold
        core_info.append((jl, rows, cnt, ch, tile_perm))

    sorted_ch = np.stack([ci[3][ci[4]] for ci in core_info])  # [NCORE, NT]
    schedule = np.maximum(sorted_ch.max(axis=0), 1).astype(np.int64)
    nchunk = int(schedule.sum())
    nload = -(-nchunk // L)
    nc_tot = nload * L

    chunk_base = np.zeros(NT + 1, dtype=np.int64)
    np.cumsum(schedule, out=chunk_base[1:])

    minj = np.full((NCORE, nc_tot), 1 << 30, dtype=np.int64)
    maxj = np.full((NCORE, nc_tot), -1, dtype=np.int64)
    fills = []
    for c in range(NCORE):
        jl, rows, cnt, ch, tile_perm = core_info[c]
        tile_start = np.zeros(NT + 1, dtype=np.int64)
        np.cumsum(cnt, out=tile_start[1:])
        ridx = np.zeros(nc_tot * P, dtype=np.int64)
        jrel = np.full(nc_tot * P, -1, dtype=np.int64)  # tile-relative
        for s in range(NT):
            T = int(tile_perm[s])
            n = int(cnt[T])
            dst = chunk_base[s] * P
            ridx[dst : dst + n] = rows[tile_start[T] : tile_start[T] + n]
            jrel[dst : dst + n] = jl[tile_start[T] : tile_start[T] + n] - T * P
        jr2 = jrel.reshape(nc_tot, P)
        valid = jr2 >= 0
        anyv = valid.any(axis=1)
        minj[c] = np.where(anyv, np.where(valid, jr2, 1 << 30).min(axis=1), 1 << 30)
        maxj[c] = np.where(anyv, np.where(valid, jr2, -1).max(axis=1), -1)
        fills.append((ridx, jr2))

    woff = np.clip(minj.min(axis=0), 0, P - W)
    is0 = np.zeros(nc_tot, dtype=bool)
    is0[chunk_base[:-1]] = True       # chunk-0 slots use full-width S0
    assert (maxj.max(axis=0)[~is0] < (woff + W)[~is0]).all(), "span > window"

    wslice = np.arange(W)[None, None, :]
    in_maps = []
    for c in range(NCORE):
        ridx, jr2 = fills[c]
        # windowed one-hot S per chunk [nc_tot, P, W] (pad rows are all-0)
        S = (jr2[:, :, None] == (woff[:, None, None] + wslice)).astype(E3)
        ea_all = ea8[ridx].reshape(nc_tot, P, D)
        rec = np.concatenate([ea_all, S], axis=2)        # [nc_tot, P, R]
        ea_pack = (
            rec.reshape(nload, L, P, R).transpose(0, 2, 1, 3)
            .reshape(nload, P, L * R).copy()
        )
        # full-width S0 for each tile's chunk 0
        jr0 = jr2[chunk_base[:-1]]                       # [NT, P]
        S0 = (jr0[:, :, None] == np.arange(P)[None, None, :]).astype(E3)
        s0_pack = (
            S0.reshape(NGRP, G, P, P).transpose(0, 2, 1, 3)
            .reshape(NGRP, P, G * P).copy()
        )

        xs = np.zeros((NPAD, D), dtype=np.float32)
        xs[:NSHARD] = x[c * NSHARD : (c + 1) * NSHARD]
        tile_perm = core_info[c][4]
        xt = xs.reshape(NT, P, D).transpose(0, 2, 1)[tile_perm]  # [NT, f, n]
        xbf_pack = (
            xt.astype(BF16).reshape(NGRP, G, D, P).transpose(0, 2, 1, 3)
            .reshape(NGRP, D, G * P).copy()
        )
        xfn = (xs + ln_b[None, :]).reshape(NT, P, D)[tile_perm]  # [NT, n, f]
        xf_pack = (
            xfn.reshape(NGRP, G, P, D).transpose(0, 2, 1, 3)
            .reshape(NGRP, P, G * D).copy()
        )

        in_maps.append(
            {
                "ea": ea_pack,
                "s0": s0_pack,
                "xbf": xbf_pack,
                "xf": xf_pack,
                "W1d": np.asarray(W1, BF16),
                "W2d": np.asarray(W2, BF16),
                "vecs": np.asarray(b1, np.float32).reshape(H, 1),
                "b2r4": np.tile(np.asarray(b2, BF16), 4).reshape(1, 4 * D),
                "gbg": np.tile(np.asarray(ln_g, np.float32), (P, G)).astype(BF16),
            }
        )

    meta = (schedule, woff, nload, nc_tot)
    return in_maps, meta, [ci[4] for ci in core_info]


def _build_program(meta):
    schedule, woff, nload, nc_tot = meta
    f32 = mybir.dt.float32
    bf16 = mybir.dt.bfloat16
    fp8 = mybir.dt.float8e3
    AF = mybir.ActivationFunctionType
    OP = mybir.AluOpType

    nc = bacc.Bacc("TRN2", target_bir_lowering=False, debug=False,
                   num_devices=NCORE)

    ea_d = nc.dram_tensor("ea", [nload, P, L * R], fp8, kind="ExternalInput").ap()
    s0_d = nc.dram_tensor("s0", [NGRP, P, G * P], fp8, kind="ExternalInput").ap()
    xbf_d = nc.dram_tensor("xbf", [NGRP, D, G * P], bf16, kind="ExternalInput").ap()
    xf_d = nc.dram_tensor("xf", [NGRP, P, G * D], f32, kind="ExternalInput").ap()
    w1_d = nc.dram_tensor("W1d", [H, H], bf16, kind="ExternalInput").ap()
    w2_d = nc.dram_tensor("W2d", [H, D], bf16, kind="ExternalInput").ap()
    vecs_d = nc.dram_tensor("vecs", [H, 1], f32, kind="ExternalInput").ap()
    b2r_d = nc.dram_tensor("b2r4", [1, 4 * D], bf16, kind="ExternalInput").ap()
    gbg_d = nc.dram_tensor("gbg", [P, G * D], bf16, kind="ExternalInput").ap()
    out_d = nc.dram_tensor("outN", [NGRP, P, G * D], f32, kind="ExternalOutput").ap()

    with tile.TileContext(nc) as tc:
        with (
            tc.tile_pool(name="consts", bufs=1) as consts,
            tc.tile_pool(name="edges", bufs=8) as epool,
            tc.tile_pool(name="s0p", bufs=3) as s0pool,
            tc.tile_pool(name="xg", bufs=3) as xpool,
            tc.tile_pool(name="yg", bufs=2) as ypool,
            tc.tile_pool(name="agg", bufs=3) as apool,
            tc.tile_pool(name="h1sb", bufs=2) as h1pool,
            tc.tile_pool(name="tb", bufs=2) as tpool,
            tc.tile_pool(name="st", bufs=2) as stpool,
            tc.tile_pool(name="psaggA", bufs=1, space="PSUM") as psaggA,
            tc.tile_pool(name="psaggB", bufs=1, space="PSUM") as psaggB,
            tc.tile_pool(name="psh1", bufs=3, space="PSUM") as psh1,
            tc.tile_pool(name="psh2", bufs=3, space="PSUM") as psh2,
        ):
            load_tiles = {}

            def ensure_load(ld):
                if ld < 0 or ld >= nload or ld in load_tiles:
                    return
                et = epool.tile([P, L * R], fp8, tag="ea", name=f"ea{ld}")
                nc.sync.dma_start(out=et[:], in_=ea_d[ld])
                load_tiles[ld] = et

            def chunk_slices(c):
                ld, sl = divmod(c, L)
                ensure_load(ld)
                ensure_load(ld + 1)
                ensure_load(ld + 2)
                t = load_tiles[ld]
                base = sl * R
                return t[:, base : base + D], t[:, base + D : base + R]

            group_state = {}

            def group_tiles(gi):
                if gi not in group_state:
                    s0g = s0pool.tile([P, G * P], fp8, tag="s0g")
                    nc.sync.dma_start(out=s0g[:], in_=s0_d[gi])
                    xb = xpool.tile([P, G * P], bf16, tag="xb")
                    nc.scalar.dma_start(out=xb[:], in_=xbf_d[gi])
                    xf = xpool.tile([P, G * D], f32, tag="xf")
                    nc.scalar.dma_start(out=xf[:], in_=xf_d[gi])
                    aggT = apool.tile([P, G * P], bf16, tag="aggT")
                    group_state[gi] = dict(s0=s0g, xb=xb, xf=xf, aggT=aggT)
                return group_state[gi]

            # prime the edge/x streams before the const DMAs so the first
            # scatter matmuls aren't stuck behind 10 small DMA issues
            group_tiles(0)
            group_tiles(1)
            for _ld in range(6):
                ensure_load(_ld)

            # ---- constants ----
            gbg_sb = consts.tile([P, G * D], bf16, tag="gbg")
            nc.scalar.dma_start(out=gbg_sb[:], in_=gbg_d[:])
            b2r4_sb = consts.tile([1, 4 * D], bf16, tag="b2r4")
            nc.scalar.dma_start(out=b2r4_sb[:], in_=b2r_d[:])
            ones_row = consts.tile([1, P], bf16, tag="ones_row")
            nc.vector.memset(ones_row[:], 1.0)
            w1xa = consts.tile([P, P], bf16, tag="w1xa")
            nc.scalar.dma_start(out=w1xa[:], in_=w1_d[0:P, 0:P])
            w1xb = consts.tile([P, P], bf16, tag="w1xb")
            nc.scalar.dma_start(out=w1xb[:], in_=w1_d[0:P, P : 2 * P])
            w1ga = consts.tile([P, P], bf16, tag="w1ga")
            nc.scalar.dma_start(out=w1ga[:], in_=w1_d[P : 2 * P, 0:P])
            w1gb = consts.tile([P, P], bf16, tag="w1gb")
            nc.scalar.dma_start(out=w1gb[:], in_=w1_d[P : 2 * P, P : 2 * P])
            w2a = consts.tile([P, P], bf16, tag="w2a")
            nc.scalar.dma_start(out=w2a[:], in_=w2_d[0:P, :])
            w2b = consts.tile([P, P], bf16, tag="w2b")
            nc.scalar.dma_start(out=w2b[:], in_=w2_d[P : 2 * P, :])
            b1a = consts.tile([P, 1], f32, tag="b1a")
            nc.scalar.dma_start(out=b1a[:], in_=vecs_d[0:P, :])
            b1b = consts.tile([P, 1], f32, tag="b1b")
            nc.scalar.dma_start(out=b1b[:], in_=vecs_d[P : 2 * P, :])
            eps_sb = consts.tile([P, 1], f32, tag="eps")
            nc.vector.memset(eps_sb[:], LN_EPS)

            chunk_base = np.zeros(NT + 1, dtype=np.int64)
            np.cumsum(schedule, out=chunk_base[1:])

            BANKS = ((0, 1, 2, 3), (4, 5, 6))

            def scatter_group(gi):
                """Scatter all 7 tiles of group gi into two shared PSUM
                banks (independent accumulation regions per tile), then one
                bank-wide PSUM->SBUF copy per bank."""
                if gi >= NGRP:
                    return
                st = group_tiles(gi)
                for bi, tiles in enumerate(BANKS):
                    pool = psaggA if bi == 0 else psaggB
                    bank = pool.tile([P, 4 * P], f32, tag="agg")
                    for k, ti in enumerate(tiles):
                        t = gi * G + ti
                        c0 = int(chunk_base[t])
                        ncch = int(schedule[t])
                        ea0, _ = chunk_slices(c0)
                        nc.tensor.matmul(
                            bank[:, k * P : (k + 1) * P], lhsT=ea0,
                            rhs=st["s0"][:, ti * P : (ti + 1) * P],
                            start=True, stop=(ncch == 1),
                            skip_group_check=True,
                        )
                        for i in range(1, ncch):
                            c = c0 + i
                            eap, sap = chunk_slices(c)
                            w = int(woff[c])
                            nc.tensor.matmul(
                                bank[:, k * P + w : k * P + w + W],
                                lhsT=eap, rhs=sap,
                                start=False, stop=(i == ncch - 1),
                                skip_group_check=True,
                            )
                    n = len(tiles)
                    dst = st["aggT"][:, tiles[0] * P : (tiles[0] + n) * P]
                    if bi == 0:
                        nc.scalar.activation(out=dst, in_=bank[:, : n * P],
                                             func=AF.Copy, bias=0.0, scale=1.0)
                    else:
                        nc.vector.tensor_copy(out=dst, in_=bank[:, : n * P])

            SB_COLS = ((0, 4 * P), (4 * P, 3 * P))

            def h1_group(gi, h1a, h1b):
                st = group_tiles(gi)
                relus = []
                for sb in (0, 1):
                    c0, cn = SB_COLS[sb]
                    psA = psh1.tile([P, 4 * P], f32, tag="h1ps",
                                    name=f"h1a{gi}_{sb}")
                    nc.tensor.matmul(psA[:, :cn], lhsT=w1xa[:],
                                     rhs=st["xb"][:, c0 : c0 + cn],
                                     start=True, stop=False)
                    nc.tensor.matmul(psA[:, :cn], lhsT=w1ga[:],
                                     rhs=st["aggT"][:, c0 : c0 + cn],
                                     start=False, stop=True)
                    psB = psh1.tile([P, 4 * P], f32, tag="h1ps",
                                    name=f"h1b{gi}_{sb}")
                    nc.tensor.matmul(psB[:, :cn], lhsT=w1xb[:],
                                     rhs=st["xb"][:, c0 : c0 + cn],
                                     start=True, stop=False)
                    nc.tensor.matmul(psB[:, :cn], lhsT=w1gb[:],
                                     rhs=st["aggT"][:, c0 : c0 + cn],
                                     start=False, stop=True)
                    # relu: a-half on ACT, b-half on DVE
                    nc.scalar.activation(out=h1a[:, c0 : c0 + cn],
                                         in_=psA[:, :cn], func=AF.Relu,
                                         bias=b1a[:], scale=1.0)
                    nc.vector.tensor_scalar(
                        out=h1b[:, c0 : c0 + cn], in0=psB[:, :cn],
                        scalar1=b1b[:], scalar2=0.0, op0=OP.add, op1=OP.max,
                    )

            def h2_bank(gi, bank, tiles, h1a, h1b):
                for k, ti in enumerate(tiles):
                    o = bank[:, k * P : (k + 1) * P]
                    sl = slice(ti * P, (ti + 1) * P)
                    nc.tensor.matmul(o, lhsT=h1a[:, sl], rhs=w2a[:],
                                     start=True, stop=False)
                    nc.tensor.matmul(o, lhsT=h1b[:, sl], rhs=w2b[:],
                                     start=False, stop=False)
                    nc.tensor.matmul(o, lhsT=ones_row[:],
                                     rhs=b2r4_sb[:, :D],
                                     start=False, stop=True)

            def ln_bank(gi, bank, tiles, stats, mv, sd, rstd, mur, nmur,
                        t1, u, y):
                st = group_tiles(gi)
                t0 = tiles[0]
                n = len(tiles)
                for k, ti in enumerate(tiles):
                    nc.vector.bn_stats(out=stats[:, 6 * ti : 6 * ti + 6],
                                       in_=bank[:, k * P : (k + 1) * P])
                for ti in tiles:
                    nc.vector.bn_aggr(out=mv[:, 2 * ti : 2 * ti + 2],
                                      in_=stats[:, 6 * ti : 6 * ti + 6])
                mv3 = mv[:].rearrange("p (t s) -> p t s", s=2)
                # last group: per-tile scalar chain so the tail doesn't wait
                # for all aggrs before starting the normalize
                steps = ([(ti, ti + 1) for ti in tiles]
                         if gi == NGRP - 1 else [(t0, t0 + n)])
                for a, b in steps:
                    nc.scalar.activation(
                        out=sd[:, a:b].rearrange("p (t s) -> p t s", s=1),
                        in_=mv3[:, a:b, 1:2], func=AF.Sqrt,
                        bias=eps_sb[:], scale=1.0,
                    )
                    nc.vector.reciprocal(out=rstd[:, a:b], in_=sd[:, a:b])
                    nc.vector.tensor_tensor(
                        out=mur[:, a:b].rearrange("p (t s) -> p t s", s=1),
                        in0=mv3[:, a:b, 0:1],
                        in1=rstd[:, a:b].rearrange("p (t s) -> p t s", s=1),
                        op=OP.mult,
                    )
                    nc.vector.tensor_scalar(
                        out=nmur[:, a:b], in0=mur[:, a:b],
                        scalar1=-1.0, scalar2=None, op0=OP.mult,
                    )
                # fused normalize (h2*rstd - mu*rstd): even tiles ACT, odd DVE
                for k, ti in enumerate(tiles):
                    src_ap = bank[:, k * P : (k + 1) * P]
                    dst_ap = t1[:, ti * D : (ti + 1) * D]
                    if ti % 2 == 0:
                        nc.scalar.activation(
                            out=dst_ap, in_=src_ap, func=AF.Identity,
                            bias=nmur[:, ti : ti + 1], scale=rstd[:, ti : ti + 1],
                        )
                    else:
                        nc.vector.tensor_scalar(
                            out=dst_ap, in0=src_ap,
                            scalar1=rstd[:, ti : ti + 1],
                            scalar2=nmur[:, ti : ti + 1],
                            op0=OP.mult, op1=OP.add,
                        )
                # gamma (Pool) + residual (DVE); all-DVE for the last group
                # to shorten the tail chain
                cs = slice(t0 * D, (t0 + n) * D)
                ln_eng = nc.vector if gi == NGRP - 1 else nc.gpsimd
                ln_eng.tensor_tensor(out=u[:, cs], in0=t1[:, cs],
                                     in1=gbg_sb[:, cs], op=OP.mult)
                ln_eng.tensor_tensor(out=y[:, cs], in0=u[:, cs],
                                     in1=st["xf"][:, cs], op=OP.add)
                out_q = nc.sync if gi == NGRP - 1 else nc.gpsimd
                out_q.dma_start(out=out_d[gi][:, cs], in_=y[:, cs])

            # ---- software pipeline: scatter runs two groups ahead ----
            scatter_group(0)
            scatter_group(1)
            for gi in range(NGRP):
                h1a = h1pool.tile([P, G * P], bf16, tag="h1a")
                h1b = h1pool.tile([P, G * P], bf16, tag="h1b")
                stats = stpool.tile([P, G * 6], f32, tag="stats")
                mv = stpool.tile([P, G * 2], f32, tag="mv")
                sd = stpool.tile([P, G], f32, tag="sd")
                rstd = stpool.tile([P, G], f32, tag="rstd")
                mur = stpool.tile([P, G], f32, tag="mur")
                nmur = stpool.tile([P, G], f32, tag="nmur")
                t1 = tpool.tile([P, G * D], bf16, tag="t1")
                u = tpool.tile([P, G * D], f32, tag="u")
                y = ypool.tile([P, G * D], f32)

                h1_group(gi, h1a, h1b)
                bankA = psh2.tile([P, 4 * P], f32, tag="h2", name=f"h2A{gi}")
                h2_bank(gi, bankA, BANKS[0], h1a, h1b)
                ln_bank(gi, bankA, BANKS[0], stats, mv, sd, rstd, mur,
                        nmur, t1, u, y)
                bankB = psh2.tile([P, 4 * P], f32, tag="h2", name=f"h2B{gi}")
                h2_bank(gi, bankB, BANKS[1], h1a, h1b)
                ln_bank(gi, bankB, BANKS[1], stats, mv, sd, rstd, mur,
                        nmur, t1, u, y)
                scatter_group(gi + 2)
                group_state.pop(gi, None)

    nc.finalize()
    return nc


LAST_RESULT = None


def kernel(x, edge_index, edge_attr, W1, b1, W2, b2, ln_g, ln_b):
    global LAST_RESULT
    in_maps, meta, tile_perms = _prep_host(
        x, edge_index, edge_attr, W1, b1, W2, b2, ln_g, ln_b
    )
    nc = _build_program(meta)
    trace = bool(os.environ.get("KERNEL_TRACE"))
    res = run_bass_kernel_spmd(
        nc, in_maps, core_ids=list(range(NCORE)), trace=trace
    )
    LAST_RESULT = res

    out = np.empty((N_NODES, D), dtype=np.float32)
    for c in range(NCORE):
        yN = res.results[c]["outN"]  # [NGRP, P, G*D] node-major, slot order
        y_slots = yN.reshape(NGRP, P, G, D).transpose(0, 2, 1, 3).reshape(NT, P, D)
        y_tiles = np.empty_like(y_slots)
        y_tiles[tile_perms[c]] = y_slots
        y = y_tiles.reshape(NPAD, D)[:NSHARD]
        out[c * NSHARD : (c + 1) * NSHARD] = y
    return out
